# revision 66
# baseline (speedup 1.0000x reference)
"""Trainium2 Bass kernel for nn_AttentionCircuit (neuron-mixture attention).

Self-contained: accepts FULL inputs, shards across 8 NeuronCores, runs a
Bass/Tile SPMD kernel, gathers the full output.

Sharding: core c = (b, g) with b = c // 4 (batch), g = c % 4 (head-group of
4 heads = 256 channels).  Features are sequence-split within each batch
group and all-gathered (q/k first, v second so restore starts earlier);
restore + attention are head-group-parallel; the output projection computes
per-core partials against the core's 256 rows of W_O (full 1024 columns)
and combines them with two ReduceScatters (heads 0-1 fired mid-attention,
heads 2-3 at the end) — no attention-output all-gather.  All TensorEngine
compute in bf16, f32 I/O and PSUM accumulation.
"""

import sys

for _p in ("/opt/trn_rl_repo",):
    if _p not in sys.path:
        sys.path.append(_p)

import numpy as np
from dataclasses import dataclass

import concourse.bass as bass
import concourse.bacc as bacc
import concourse.mybir as mybir
import concourse.tile as tile
from concourse import masks
from concourse.bass_utils import run_bass_kernel_spmd

try:
    import ml_dtypes

    BF16 = ml_dtypes.bfloat16
except ImportError:  # pragma: no cover
    BF16 = np.float32


def _install_neff_disk_cache():
    """Cache walrus BIR->NEFF compiles on disk (keyed by BIR bytes) so
    repeated runs of the identical graph skip the multi-minute compile."""
    import hashlib, os, tempfile
    from concourse import bass2jax

    if getattr(bass2jax, "_ant_neff_cache_installed", False):
        return
    orig = bass2jax.compile_bir_kernel
    cache_dir = os.path.join(tempfile.gettempdir(), "bass_neff_cache")
    os.makedirs(cache_dir, exist_ok=True)

    def cached(bir_json, tmpdir, neff_name="file.neff"):
        key = hashlib.sha256(bir_json).hexdigest()
        path = os.path.join(cache_dir, key + ".neff")
        dst = os.path.join(tmpdir, neff_name)
        if os.path.exists(path):
            import shutil

            shutil.copy(path, dst)
            return dst
        neff = orig(bir_json, tmpdir, neff_name=neff_name)
        try:
            import shutil

            shutil.copy(neff, path)
        except OSError:
            pass
        return neff

    bass2jax.compile_bir_kernel = cached
    bass2jax._ant_neff_cache_installed = True


_install_neff_disk_cache()

F32 = mybir.dt.float32
BF = mybir.dt.bfloat16
P = 128  # partitions


@dataclass(frozen=True)
class Cfg:
    B: int = 2
    S: int = 2048
    D: int = 1024
    R: int = 128
    N: int = 8
    H: int = 16
    cores: int = 8

    @property
    def G(self):  # cores per batch == head groups
        return self.cores // self.B

    @property
    def S_sl(self):  # sequence slice per core (feature stage)
        return self.S // self.G

    @property
    def COLS(self):  # channel columns per core
        return self.D // self.G

    @property
    def Hpc(self):  # heads per core
        return self.H // self.G

    @property
    def dh(self):
        return self.D // self.H

    @property
    def KD(self):  # k-tiles over D
        return self.D // P

    @property
    def NR(self):
        return self.N * self.R

    @property
    def KNR(self):  # k-tiles over N*R
        return self.NR // P

    @property
    def ST(self):  # s-tiles over full S
        return self.S // P

    @property
    def ST_sl(self):  # s-tiles over S slice
        return self.S_sl // P

    @property
    def CT(self):  # 128-col tiles over COLS
        return (self.COLS + P - 1) // P


FULL = Cfg()


def _ceil_div(a, b):
    return (a + b - 1) // b


def build_nc(cfg: Cfg = FULL, fake_cc: bool = False, debug_taps: bool = False) -> bacc.Bacc:
    """Build + compile the SPMD graph (identical on every core).

    fake_cc=True replaces collectives with local DMA (wrong results) for
    single-core debug; the TimelineSim cost model prices real collectives
    directly, so the default graph is used for both sim and hardware.
    """
    c = cfg
    assert c.R == P and c.D % P == 0 and c.S_sl % P == 0
    assert P % c.dh == 0 and c.COLS % c.dh == 0 and c.COLS % P == 0

    nc = bacc.Bacc(
        "TRN2",
        target_bir_lowering=False,
        debug=False,
        num_devices=1 if fake_cc else c.cores,
    )

    group0 = list(range(c.G))
    group1 = list(range(c.G, 2 * c.G))
    rgroups = [group0, group1]

    def all_gather(in_ap, out_t):
        if fake_cc:
            for g in range(c.G):
                nc.sync.dma_start(out=out_t.ap()[g], in_=in_ap)
        else:
            nc.gpsimd.collective_compute(
                "AllGather",
                mybir.AluOpType.bypass,
                replica_groups=rgroups,
                ins=[in_ap.opt()],
                outs=[out_t.ap().opt()],
            )

    def reduce_scatter(in_ap, out_ap):
        if fake_cc:
            nc.sync.dma_start(out=out_ap, in_=in_ap[0])
        else:
            nc.gpsimd.collective_compute(
                "ReduceScatter",
                mybir.AluOpType.add,
                replica_groups=rgroups,
                ins=[in_ap.opt()],
                outs=[out_ap.opt()],
            )

    scale = 1.0 / float(np.sqrt(c.dh))
    DHO = c.dh + 1  # dh + ones column

    # ---- DRAM parameters (host-prepped layouts, see shard_inputs) ----
    xT = nc.dram_tensor("xT", [P, c.KD, c.S_sl], BF, kind="ExternalInput")
    fqk = nc.dram_tensor("fqk", [P, c.KD, c.NR], BF, kind="ExternalInput")
    fv = nc.dram_tensor("fv", [P, c.KD, c.NR], BF, kind="ExternalInput")
    rqk = nc.dram_tensor("rqk", [P, c.KNR, c.COLS], BF, kind="ExternalInput")
    rv = nc.dram_tensor("rv", [P, c.KNR, c.COLS], BF, kind="ExternalInput")
    # W_O row-shard: this core's 256 rows x all 1024 cols -> [P, CT, D]
    wo = nc.dram_tensor("wo", [P, c.CT, c.D], BF, kind="ExternalInput")
    # combine scalars (feature weights for this core's s-slice), f32
    wcomb = nc.dram_tensor("wcomb", [P, 3, c.ST_sl, c.N], F32, kind="ExternalInput")
    # restore weights, full S, bf16 (broadcast source): [3N, S]
    wsm = nc.dram_tensor("wsm", [3 * c.N, c.S], BF, kind="ExternalInput")
    out_d = nc.dram_tensor("out", [c.S, c.COLS], F32, kind="ExternalOutput")
    if debug_taps:
        dbg_qT = nc.dram_tensor("dbg_qT", [P, c.CT, c.S], F32, kind="ExternalOutput")
        dbg_kT = nc.dram_tensor("dbg_kT", [P, c.CT, c.S], F32, kind="ExternalOutput")
        dbg_v = nc.dram_tensor("dbg_v", [P, c.ST, c.Hpc * (c.dh + 1)], F32, kind="ExternalOutput")
        dbg_at = nc.dram_tensor("dbg_at", [P, c.CT, c.S], F32, kind="ExternalOutput")
        dbg_attn = nc.dram_tensor("dbg_attn", [P, c.ST, c.Hpc * c.dh], F32, kind="ExternalOutput")
        dbg_part = nc.dram_tensor("dbg_part", [c.G, c.S // 2, c.COLS], BF, kind="ExternalOutput")

    # collective bounce buffers
    hT_all_in = nc.dram_tensor("hT_all_in", [3, P, c.S_sl], BF)
    hT_all_out = nc.dram_tensor("hT_all_out", [c.G, 3, P, c.S_sl], BF)
    part_d = nc.dram_tensor("part_d", [c.G, c.S, c.COLS], BF)
    rs_d = nc.dram_tensor("rs_d", [c.S, c.COLS], BF)

    from contextlib import ExitStack

    with tile.TileContext(nc) as tc, ExitStack() as stack:
        # ------- constants -------
        const_pool = stack.enter_context(tc.tile_pool(name="const", bufs=1))
        ident = const_pool.tile([P, P], BF)
        masks.make_identity(nc, ident[:])
        cmask = const_pool.tile([P, P], BF)
        masks.make_upper_triangular(nc, cmask[:], val=1.0, diag=True)

        # ------- long-lived SBUF residents -------
        res_pool = stack.enter_context(tc.tile_pool(name="residents", bufs=1))
        rqk_sb = res_pool.tile([P, c.KNR, c.COLS], BF)
        rv_sb = res_pool.tile([P, c.KNR, c.COLS], BF)
        wo_sb = res_pool.tile([P, c.CT, c.D], BF)
        wcomb_sb = res_pool.tile([P, 3, c.ST_sl, c.N], F32)
        nc.sync.dma_start(out=wcomb_sb[:], in_=wcomb[:])

        qT_ct = [
            res_pool.tile([P, c.S], BF, name=f"qT{i}") for i in range(c.CT)
        ]
        kT_ct = [
            res_pool.tile([P, c.S], BF, name=f"kT{i}") for i in range(c.CT)
        ]
        v_sb = res_pool.tile([P, c.ST, c.Hpc * DHO], BF)
        attn_sb = res_pool.tile([P, c.ST, c.Hpc * c.dh], BF)
        aT_sb = res_pool.tile([P, c.CT, c.S], BF)
        # rows [1, S] of the v restore weights (pool-broadcast source)
        wst_sb = res_pool.tile([c.N, c.S], BF)
        nc.sync.dma_start(out=wst_sb[:], in_=wsm.ap()[2 * c.N : 3 * c.N, :])

        # ones columns of v_sb
        v4 = v_sb[:].rearrange("p st (h x) -> p st h x", x=DHO)
        nc.gpsimd.memset(v4[:, :, :, c.dh : c.dh + 1], 1.0)

        # ================= Stage A: features on the s-slice =================
        with (
            tc.tile_pool(name="featA", bufs=2) as fpool,
            tc.tile_pool(name="featP", bufs=6, space="PSUM") as fps_pool,
            tc.tile_pool(name="featH", bufs=2) as hpool,
            tc.tile_pool(name="featHT", bufs=2, space="PSUM") as htps_pool,
        ):
            xT_sb = fpool.tile([P, c.KD, c.S_sl], BF, tag="xT", bufs=1)
            fqk_sb = fpool.tile([P, c.KD, c.NR], BF, tag="fqk", bufs=1)
            fv_sb = fpool.tile([P, c.KD, c.NR], BF, tag="fv", bufs=1)
            # per-k loads pipeline with the k-accumulation of the first
            # feature matmuls; fv is only needed in pass 2
            for k in range(c.KD):
                nc.sync.dma_start(out=xT_sb[:, k, :], in_=xT[:, k, :])
                nc.sync.dma_start(out=fqk_sb[:, k, :], in_=fqk[:, k, :])
            for k in range(c.KD):
                nc.sync.dma_start(out=fv_sb[:, k, :], in_=fv[:, k, :])

            # staging for h^T: copies land per s-tile, one DMA per tensor
            hT_stage = [
                hpool.tile([P, c.S_sl], BF, tag=f"hTs{t}", name=f"hT_stage{t}")
                for t in range(3)
            ]

            f_chunk = min(c.NR, 512)
            n_ch = _ceil_div(c.NR, f_chunk)
            n_per_ch = f_chunk // c.R

            def combine(t, st, ah_tiles):
                # h[s, r] = sum_n w[s, n] * all_h[s, n*R+r], 2-byte SBUF mode
                eng = nc.vector
                h_t = hpool.tile([P, c.R], BF, tag="hacc")
                for n in range(c.N):
                    ah = ah_tiles[n // n_per_ch]
                    src = ah[:, c.R * (n % n_per_ch) : c.R * (n % n_per_ch + 1)]
                    if n == 0:
                        eng.tensor_scalar(
                            out=h_t[:],
                            in0=src,
                            scalar1=wcomb_sb[:, t, st, 0:1],
                            scalar2=None,
                            op0=mybir.AluOpType.mult,
                        )
                    else:
                        eng.scalar_tensor_tensor(
                            out=h_t[:],
                            in0=src,
                            scalar=wcomb_sb[:, t, st, n : n + 1],
                            in1=h_t[:],
                            op0=mybir.AluOpType.mult,
                            op1=mybir.AluOpType.add,
                        )
                htp = htps_pool.tile([P, P], BF, tag="htp")
                nc.tensor.transpose(htp[:], h_t[:], ident[:])
                nc.scalar.copy(hT_stage[t][:, P * st : P * (st + 1)], htp[:, :])

            # pass 1: q/k features only, so their AllGather fires early;
            # pass 2 (v) runs under the q/k AllGather
            for pi, f_sb in ((0, fqk_sb), (1, fv_sb)):
                for st in range(c.ST_sl):
                    ah_tiles = []
                    for ch in range(n_ch):
                        ps = fps_pool.tile([P, f_chunk], F32, tag="feat")
                        lo = f_chunk * ch
                        hi = min(c.NR, lo + f_chunk)
                        for k in range(c.KD):
                            nc.tensor.matmul(
                                ps[:, 0 : hi - lo],
                                lhsT=xT_sb[:, k, P * st : P * (st + 1)],
                                rhs=f_sb[:, k, lo:hi],
                                start=(k == 0),
                                stop=(k == c.KD - 1),
                            )
                        ah = hpool.tile([P, f_chunk], BF, tag="ah", bufs=4)
                        nc.scalar.copy(ah[:], ps[:])
                        ah_tiles.append(ah)
                    if pi == 0:
                        combine(0, st, ah_tiles)
                        combine(1, st, ah_tiles)
                    else:
                        combine(2, st, ah_tiles)
                if pi == 0:
                    nc.sync.dma_start(out=hT_all_in[0, :, :], in_=hT_stage[0][:])
                    nc.sync.dma_start(out=hT_all_in[1, :, :], in_=hT_stage[1][:])
                else:
                    nc.sync.dma_start(out=hT_all_in[2, :, :], in_=hT_stage[2][:])
                    all_gather(hT_all_in.ap()[:], hT_all_out)

        # g tiles and q/k weight-row broadcasts: pools open once stage A's
        # SBUF is released; the broadcast DMAs run while the AllGather is in
        # flight.  One partition-broadcast DMA per tensor.
        g_pool = stack.enter_context(tc.tile_pool(name="g", bufs=16))
        g_tiles = {}
        wrep_stack = ExitStack()
        wrep_pool = wrep_stack.enter_context(tc.tile_pool(name="wrep", bufs=2))
        wr_full = {}
        for t in (0, 1):
            wrt = wrep_pool.tile([P, c.N, c.S], BF, tag="wrep", name=f"wr_t{t}")
            nc.sync.dma_start(
                out=wrt[:],
                in_=wsm.ap()[t * c.N : (t + 1) * c.N, :]
                .unsqueeze(0)
                .broadcast_to([P, c.N, c.S]),
            )
            wr_full[t] = wrt

        # deferred resident loads (consumed by restore / output projection)
        nc.sync.dma_start(out=rqk_sb[:], in_=rqk[:])
        nc.sync.dma_start(out=rv_sb[:], in_=rv[:])
        nc.sync.dma_start(out=wo_sb[:], in_=wo[:])

        # hT_full[r, g, t, s_in]  (s blocked by source rank g), batched DMAs
        hT_sb = res_pool.tile([P, c.G, 3, c.S_sl], BF)
        for t in range(3):
            nc.sync.dma_start(
                out=hT_sb[:, :, t, :],
                in_=hT_all_out.ap()[:, t, :, :].rearrange("g p s -> p g s"),
            )

        # ======= Stage C1: g tiles for q/k + Q^T/K^T restore (ct 0) =======
        n_sch = _ceil_div(c.S, 512)

        def qk_restore_chunk(pool, t, ct, ch, copy_eng):
            lo, hi = 512 * ch, min(c.S, 512 * ch + 512)
            dst = qT_ct[ct] if t == 0 else kT_ct[ct]
            rps = pool.tile([P, 512], F32, tag="rps")
            for n in range(c.KNR):
                nc.tensor.matmul(
                    rps[:, 0 : hi - lo],
                    lhsT=rqk_sb[:, n, P * ct : P * (ct + 1)],
                    rhs=g_tiles[(t, n)][:, lo:hi],
                    start=(n == 0),
                    stop=(n == c.KNR - 1),
                )
            copy_eng(dst[:, lo:hi], rps[:, 0 : hi - lo])

        with tc.tile_pool(name="rps0", bufs=2, space="PSUM") as rps0_pool:
            for t in (0, 1):
                for n in range(c.N):
                    g_tiles[(t, n)] = g_pool.tile(
                        [P, c.S], BF, tag="g", name=f"g_{t}_{n}"
                    )
            for ch in range(n_sch):
                lo, hi = 512 * ch, 512 * ch + 512
                for t in (0, 1):
                    for n in range(c.N):
                        eng = nc.vector if n < 6 else nc.gpsimd
                        eng.tensor_mul(
                            g_tiles[(t, n)][:, lo:hi],
                            hT_sb[:, ch, t, :],
                            wr_full[t][:, n, lo:hi],
                        )
                    qk_restore_chunk(rps0_pool, t, 0, ch, nc.scalar.copy)
        wrep_stack.close()  # q/k weight rows dead once g built

        # ================= Stage D: causal attention per head =================
        # per-j probs tiles sized to the causal width; j 0/1 double-buffered so
        # the next head's scores can start while this head's AVs drain
        pr_pool = stack.enter_context(tc.tile_pool(name="probs", bufs=1))
        asm_pool = stack.enter_context(tc.tile_pool(name="attn_small", bufs=4))
        atps_pool = stack.enter_context(
            tc.tile_pool(name="atps", bufs=1, space="PSUM")
        )
        av_pool = stack.enter_context(
            tc.tile_pool(name="avps", bufs=1, space="PSUM")
        )
        sps_stack = ExitStack()
        sps_pool = sps_stack.enter_context(
            tc.tile_pool(name="sps", bufs=2, space="PSUM")
        )
        late_stack = ExitStack()

        SCH = 1024  # scores chunk (2 PSUM banks); exp whole chunk

        def head_scores(h, js, probs):
            ct = (c.dh * h) // P
            off = (c.dh * h) % P
            for j in js:
                qlo = P * j
                qn = c.S - qlo
                pj = pr_pool.tile(
                    [P, qn], BF, tag=f"pj{j}", name=f"pj_{j}",
                    bufs=2 if j < 8 else 1,
                )
                probs.append(pj)
                for chx in range(_ceil_div(qn, SCH)):
                    lo = qlo + SCH * chx
                    hi = min(c.S, lo + SCH)
                    sps = sps_pool.tile([P, SCH], F32, tag="sps")
                    for sub in range(_ceil_div(hi - lo, 512)):
                        slo, shi = lo + 512 * sub, min(hi, lo + 512 * sub + 512)
                        nc.tensor.matmul(
                            sps[:, slo - lo : shi - lo],
                            lhsT=kT_ct[ct][off : off + c.dh, qlo : qlo + P],
                            rhs=qT_ct[ct][off : off + c.dh, slo:shi],
                            start=True,
                            stop=True,
                        )
                    nc.scalar.activation(
                        pj[:, lo - qlo : hi - qlo],
                        sps[:, 0 : hi - lo],
                        mybir.ActivationFunctionType.Exp,
                        scale=scale,
                    )
                # mask the diagonal tile (keep q >= k); Pool op frees DVE
                nc.gpsimd.tensor_mul(pj[:, 0:P], pj[:, 0:P], cmask[:])

        def head_av(h, probs, j, extra=None):
            av = av_pool.tile([P, DHO], F32, tag="av")
            for j2 in range(j + 1):
                nc.tensor.matmul(
                    av[:, :],
                    lhsT=probs[j2][:, P * (j - j2) : P * (j - j2) + P],
                    rhs=v_sb[:, j2, DHO * h : DHO * (h + 1)],
                    start=(j2 == 0),
                    stop=(j2 == j),
                )
            rec = asm_pool.tile([P, 1], F32, tag="rec")
            nc.vector.reciprocal(rec[:], av[:, c.dh : c.dh + 1])
            nc.vector.tensor_scalar(
                out=attn_sb[:, j, c.dh * h : c.dh * (h + 1)],
                in0=av[:, 0 : c.dh],
                scalar1=rec[:],
                scalar2=None,
                op0=mybir.AluOpType.mult,
            )

        # ---- pipelined attention schedule: each next head's first score
        # tiles are emitted before the current head's AVs (their pj tiles are
        # double-buffered) so the exp chain never starves at head boundaries;
        # V restore + QK ct-1 + g(v) fill the other engines under head 0 ----
        probs = [[] for _ in range(c.Hpc)]
        head_scores(0, range(c.ST), probs[0])

        # QK restore ct 1 (copies on DVE: Act is busy with exp)
        with tc.tile_pool(name="rps1", bufs=2, space="PSUM") as rps1_pool:
            for t in (0, 1):
                for ch in range(n_sch):
                    qk_restore_chunk(rps1_pool, t, 1, ch, nc.vector.tensor_copy)

        head_scores(1, range(0, 8), probs[1])

        # g tiles for v: rows broadcast on Pool, mults on DVE.  These reuse
        # the q g-tile slots, whose last readers are the ct-1 matmuls above —
        # so this section must stay after ct-1 in PE program order.
        with tc.tile_pool(name="wrb", bufs=3) as wrb_pool:
            wrbs = []
            for n in range(c.N):
                wrb = wrb_pool.tile([P, c.S], BF, tag="wrb", name=f"wrb{n}")
                nc.sync.dma_start(
                    out=wrb[:],
                    in_=wsm.ap()[2 * c.N + n : 2 * c.N + n + 1, :].broadcast_to(
                        [P, c.S]
                    ),
                )
                wrbs.append(wrb)
            for n in range(c.N):
                g_tiles[(2, n)] = g_pool.tile(
                    [P, c.S], BF, tag="g", name=f"g_2_{n}"
                )
                eng = nc.vector if n % 2 == 0 else nc.gpsimd
                eng.tensor_mul(
                    g_tiles[(2, n)][:].rearrange("p (g s) -> p g s", g=c.G),
                    hT_sb[:, :, 2, :],
                    wrbs[n][:].rearrange("p (g s) -> p g s", g=c.G),
                )

        # V restore interleaved with head-0 AV (scatter copies on DVE)
        with tc.tile_pool(name="vps", bufs=2, space="PSUM") as vps_pool:
            for st in range(c.ST):
                vps = vps_pool.tile([P, c.COLS], F32, tag="vps")
                for n in range(c.KNR):
                    nc.tensor.matmul(
                        vps[:, :],
                        lhsT=g_tiles[(2, n)][:, P * st : P * (st + 1)],
                        rhs=rv_sb[:, n, :],
                        start=(n == 0),
                        stop=(n == c.KNR - 1),
                    )
                nc.vector.tensor_copy(
                    v4[:, st, :, 0 : c.dh],
                    vps[:, :].rearrange("p (h x) -> p h x", x=c.dh),
                )
                head_av(0, probs[0], st)

        # late pools: output-projection partial staging
        po_pool = late_stack.enter_context(tc.tile_pool(name="po", bufs=2))
        fin_stack = ExitStack()
        fin_pool = fin_stack.enter_context(tc.tile_pool(name="fin", bufs=2))
        FB = c.ST // 8  # s-tiles per convert batch

        def fin_convert(quarter, after=None):
            # bf16 reduce-scatter result -> f32 output rows; DMAs issue from
            # the Activation sequencer so they can't head-of-line-block the
            # partial-out DMAs still flowing on the SP queue
            lo = quarter * FB * P
            a_sb = fin_pool.tile([P, FB, c.COLS], BF, tag="fa", name="fa")
            rd = nc.scalar.dma_start(
                out=a_sb[:],
                in_=rs_d.ap()[lo : lo + FB * P, :].rearrange(
                    "(b p) k -> p b k", p=P
                ),
            )
            # keep these DMAs behind the last partial-out DMA: scheduled
            # earlier, they interleave their HWDGE sem procs with the
            # partial-out DMAs and FIFO-chain them behind the reduce-scatter
            if after is not None:
                bass._add_dep_helper(
                    rd.ins, after.ins, sync=True, reason="fin after partials"
                )
            o_sb = fin_pool.tile([P, FB, c.COLS], F32, tag="fo", name="fo")
            nc.scalar.copy(o_sb[:], a_sb[:])
            olo = quarter * FB * P
            wr = nc.scalar.dma_start(
                out=out_d.ap()[olo : olo + FB * P, :].rearrange(
                    "(b p) k -> p b k", p=P
                ),
                in_=o_sb[:],
            )


        def at_transpose(ct2, st):
            # attn^T tile for (ct2, st) into the resident aT buffer
            atp = atps_pool.tile([P, P], BF, tag="atp")
            nc.tensor.transpose(
                atp[:, :], attn_sb[:, st, P * ct2 : P * (ct2 + 1)], ident[:]
            )
            nc.vector.tensor_copy(aT_sb[:, ct2, P * st : P * (st + 1)], atp[:, :])

        def wo_partial(wops_pool, st, copy_engs):
            # full partial out rows for this st: contraction over all 256 own
            # d-rows (both column tiles).  Halves go to separate DRAM tensors
            # so the first ReduceScatter doesn't WAR-block later writes.
            po = po_pool.tile([P, c.D], BF, tag="po", name="po")
            for chx in range(2):
                wops = wops_pool.tile([P, 512], F32, tag="wops")
                for kd in range(c.CT):
                    nc.tensor.matmul(
                        wops[:, :],
                        lhsT=aT_sb[:, kd, P * st : P * (st + 1)],
                        rhs=wo_sb[:, kd, 512 * chx : 512 * (chx + 1)],
                        start=(kd == 0),
                        stop=(kd == c.CT - 1),
                    )
                copy_engs[chx](po[:, 512 * chx : 512 * (chx + 1)], wops[:, :])
            return nc.sync.dma_start(
                out=part_d.ap()[:, P * st : P * (st + 1), :].rearrange(
                    "g p k -> p g k"
                ),
                in_=po[:].rearrange("p (g k) -> p g k", g=c.G),
            )

        # ---- heads 1-3, software-pipelined ----
        head_scores(1, range(8, c.ST), probs[1])
        head_scores(2, range(0, 8), probs[2])
        for j in range(c.ST):
            head_av(1, probs[1], j)
            at_transpose(0, j)
        head_scores(2, range(8, c.ST), probs[2])
        head_scores(3, range(0, 8), probs[3])
        for j in range(c.ST):
            head_av(2, probs[2], j)
        head_scores(3, range(8, c.ST), probs[3])
        SH = c.ST // 2
        with tc.tile_pool(name="wops", bufs=2, space="PSUM") as wops_pool:
            for j in range(c.ST):
                head_av(3, probs[3], j)
                at_transpose(1, j)
                if j > 0:
                    ce = nc.scalar.copy if j - 1 >= 8 else nc.vector.tensor_copy
                    wo_partial(wops_pool, j - 1, (ce, nc.vector.tensor_copy))
            last_pdma = wo_partial(
                wops_pool, c.ST - 1,
                (nc.scalar.copy, nc.vector.tensor_copy),
            )
        sps_stack.close()
        reduce_scatter(part_d.ap()[:], rs_d.ap()[:])
        for q_ in range(8):
            fin_convert(q_, last_pdma)
        if debug_taps:
            with tc.tile_pool(name="dbg", bufs=1) as dbg_pool:
                def dump(sb_flat, dr_flat, total):
                    for lo in range(0, total, 512):
                        hi = min(total, lo + 512)
                        d_sb = dbg_pool.tile([P, 512], F32, tag="d", name="dch")
                        nc.vector.tensor_copy(d_sb[:, 0 : hi - lo], sb_flat[:, lo:hi])
                        nc.sync.dma_start(out=dr_flat[:, lo:hi], in_=d_sb[:, 0 : hi - lo])

                dump(qT_ct[0][:], dbg_qT.ap()[:, 0, :], c.S)
                dump(qT_ct[1][:], dbg_qT.ap()[:, 1, :], c.S)
                dump(kT_ct[0][:], dbg_kT.ap()[:, 0, :], c.S)
                dump(kT_ct[1][:], dbg_kT.ap()[:, 1, :], c.S)
                dump(v_sb[:].rearrange("p a b -> p (a b)"),
                     dbg_v.ap().rearrange("p a b -> p (a b)"), c.ST * c.Hpc * DHO)
                dump(attn_sb[:].rearrange("p a b -> p (a b)"),
                     dbg_attn.ap().rearrange("p a b -> p (a b)"), c.ST * c.Hpc * c.dh)
                dump(aT_sb[:, 0, :], dbg_at.ap()[:, 0, :], c.S)
                dump(aT_sb[:, 1, :], dbg_at.ap()[:, 1, :], c.S)
                # part half 0 raw (bf16 DRAM->DRAM via SBUF bounce)
                for blk in range(c.S // 2 // P):
                    pb = dbg_pool.tile([P, c.G, c.COLS], BF, tag="pb", name="pb")
                    nc.sync.dma_start(
                        out=pb[:],
                        in_=part_d.ap()[:, P * blk : P * (blk + 1), :].rearrange(
                            "g p k -> p g k"
                        ),
                    )
                    nc.sync.dma_start(
                        out=dbg_part.ap()[:, P * blk : P * (blk + 1), :].rearrange(
                            "g p k -> p g k"
                        ),
                        in_=pb[:],
                    )
        fin_stack.close()
        late_stack.close()

    nc.compile()
    return nc


# ---------------------------------------------------------------------------
# Host-side sharding / gathering
# ---------------------------------------------------------------------------


def shard_inputs(
    inputs: dict,
    cfg: Cfg = FULL,
) -> list[dict]:
    c = cfg
    x = np.asarray(inputs["x"], np.float32)
    fqk_n = np.asarray(inputs["f_qk_neurons"], np.float32)
    fv_n = np.asarray(inputs["f_v_neurons"], np.float32)
    rqk_n = np.asarray(inputs["r_qk_neurons"], np.float32)
    rv_n = np.asarray(inputs["r_v_neurons"], np.float32)
    w_o = np.asarray(inputs["W_O"], np.float32)

    def tile_p(a, kt):  # [D, M] -> [P, kt, M]
        d, m = a.shape
        assert d == kt * P
        return np.ascontiguousarray(a.reshape(kt, P, m).transpose(1, 0, 2))

    # [N, D, R] -> [D, N*R]
    f_qk_flat = fqk_n.transpose(1, 0, 2).reshape(c.D, c.NR)
    f_v_flat = fv_n.transpose(1, 0, 2).reshape(c.D, c.NR)
    # [N, R, D] -> [N*R, D]
    r_qk_flat = rqk_n.reshape(c.NR, c.D)
    r_v_flat = rv_n.reshape(c.NR, c.D)

    in_maps = []
    for core in range(c.cores):
        b, g = core // c.G, core % c.G
        sl = slice(c.S_sl * g, c.S_sl * (g + 1))
        cols = slice(c.COLS * g, c.COLS * (g + 1))
        rows = slice(c.COLS * g, c.COLS * (g + 1))

        xT = x[b].T[:, sl]  # [D, S_sl]

        wq = np.asarray(inputs["fqk_weights_Q"], np.float32)[b, sl]  # [S_sl, N]
        wk = np.asarray(inputs["fqk_weights_K"], np.float32)[b, sl]
        wv = np.asarray(inputs["fv_weights"], np.float32)[b, sl]
        wcomb = np.stack([wq, wk, wv], 0)  # [3, S_sl, N]
        wcomb = np.ascontiguousarray(
            wcomb.reshape(3, c.ST_sl, P, c.N).transpose(2, 0, 1, 3)
        )  # [P, 3, ST_sl, N]

        wsm = np.stack(
            [
                np.asarray(inputs["rqk_weights_Q"], np.float32)[b].T,
                np.asarray(inputs["rqk_weights_K"], np.float32)[b].T,
                np.asarray(inputs["rv_weights"], np.float32)[b].T,
            ],
            0,
        ).reshape(3 * c.N, c.S)  # [3N, S]

        m = {
            "xT": tile_p(xT, c.KD).astype(BF16),
            "fqk": tile_p(f_qk_flat, c.KD).astype(BF16),
            "fv": tile_p(f_v_flat, c.KD).astype(BF16),
            "rqk": tile_p(r_qk_flat[:, cols], c.KNR).astype(BF16),
            "rv": tile_p(r_v_flat[:, cols], c.KNR).astype(BF16),
            "wo": tile_p(w_o[rows, :], c.CT).astype(BF16),
            "wcomb": wcomb.astype(np.float32),
            "wsm": wsm.astype(BF16),
        }
        in_maps.append(m)
    return in_maps


def gather_output(results: list[dict], cfg: Cfg = FULL) -> np.ndarray:
    c = cfg
    out = np.empty((c.B, c.S, c.D), np.float32)
    for core in range(c.cores):
        b, g = core // c.G, core % c.G
        out[b, :, c.COLS * g : c.COLS * (g + 1)] = np.asarray(
            results[core]["out"], np.float32
        )
    return out


_NC_CACHE = {}


def get_nc(cfg: Cfg = FULL) -> bacc.Bacc:
    if cfg not in _NC_CACHE:
        _NC_CACHE[cfg] = build_nc(cfg)
    return _NC_CACHE[cfg]


def kernel(**inputs) -> np.ndarray:
    cfg = FULL
    nc = get_nc(cfg)
    in_maps = shard_inputs(inputs, cfg)
    res = run_bass_kernel_spmd(nc, in_maps, core_ids=list(range(cfg.cores)))
    return gather_output(res.results, cfg)


# revision 71
# speedup vs baseline: 1.0113x; 1.0113x over previous
"""Trainium2 Bass kernel for nn_AttentionCircuit (neuron-mixture attention).

Self-contained: accepts FULL inputs, shards across 8 NeuronCores, runs a
Bass/Tile SPMD kernel, gathers the full output.

Sharding: core c = (b, g) with b = c // 4 (batch), g = c % 4 (head-group of
4 heads = 256 channels).  Features are sequence-split within each batch
group and all-gathered (q/k first, v second so restore starts earlier);
restore + attention are head-group-parallel; the output projection computes
per-core partials against the core's 256 rows of W_O (full 1024 columns)
and combines them with two ReduceScatters (heads 0-1 fired mid-attention,
heads 2-3 at the end) — no attention-output all-gather.  All TensorEngine
compute in bf16, f32 I/O and PSUM accumulation.
"""

import sys

for _p in ("/opt/trn_rl_repo",):
    if _p not in sys.path:
        sys.path.append(_p)

import numpy as np
from dataclasses import dataclass

import concourse.bass as bass
import concourse.bacc as bacc
import concourse.mybir as mybir
import concourse.tile as tile
from concourse import masks
from concourse.bass_utils import run_bass_kernel_spmd

try:
    import ml_dtypes

    BF16 = ml_dtypes.bfloat16
except ImportError:  # pragma: no cover
    BF16 = np.float32


def _install_neff_disk_cache():
    """Cache walrus BIR->NEFF compiles on disk (keyed by BIR bytes) so
    repeated runs of the identical graph skip the multi-minute compile."""
    import hashlib, os, tempfile
    from concourse import bass2jax

    if getattr(bass2jax, "_ant_neff_cache_installed", False):
        return
    orig = bass2jax.compile_bir_kernel
    cache_dir = os.path.join(tempfile.gettempdir(), "bass_neff_cache")
    os.makedirs(cache_dir, exist_ok=True)

    def cached(bir_json, tmpdir, neff_name="file.neff"):
        key = hashlib.sha256(bir_json).hexdigest()
        path = os.path.join(cache_dir, key + ".neff")
        dst = os.path.join(tmpdir, neff_name)
        if os.path.exists(path):
            import shutil

            shutil.copy(path, dst)
            return dst
        neff = orig(bir_json, tmpdir, neff_name=neff_name)
        try:
            import shutil

            shutil.copy(neff, path)
        except OSError:
            pass
        return neff

    bass2jax.compile_bir_kernel = cached
    bass2jax._ant_neff_cache_installed = True


_install_neff_disk_cache()

F32 = mybir.dt.float32
BF = mybir.dt.bfloat16
P = 128  # partitions


@dataclass(frozen=True)
class Cfg:
    B: int = 2
    S: int = 2048
    D: int = 1024
    R: int = 128
    N: int = 8
    H: int = 16
    cores: int = 8

    @property
    def G(self):  # cores per batch == head groups
        return self.cores // self.B

    @property
    def S_sl(self):  # sequence slice per core (feature stage)
        return self.S // self.G

    @property
    def COLS(self):  # channel columns per core
        return self.D // self.G

    @property
    def Hpc(self):  # heads per core
        return self.H // self.G

    @property
    def dh(self):
        return self.D // self.H

    @property
    def KD(self):  # k-tiles over D
        return self.D // P

    @property
    def NR(self):
        return self.N * self.R

    @property
    def KNR(self):  # k-tiles over N*R
        return self.NR // P

    @property
    def ST(self):  # s-tiles over full S
        return self.S // P

    @property
    def ST_sl(self):  # s-tiles over S slice
        return self.S_sl // P

    @property
    def CT(self):  # 128-col tiles over COLS
        return (self.COLS + P - 1) // P


FULL = Cfg()


def _ceil_div(a, b):
    return (a + b - 1) // b


def build_nc(cfg: Cfg = FULL, fake_cc: bool = False, debug_taps: bool = False) -> bacc.Bacc:
    """Build + compile the SPMD graph (identical on every core).

    fake_cc=True replaces collectives with local DMA (wrong results) for
    single-core debug; the TimelineSim cost model prices real collectives
    directly, so the default graph is used for both sim and hardware.
    """
    c = cfg
    assert c.R == P and c.D % P == 0 and c.S_sl % P == 0
    assert P % c.dh == 0 and c.COLS % c.dh == 0 and c.COLS % P == 0

    nc = bacc.Bacc(
        "TRN2",
        target_bir_lowering=False,
        debug=False,
        num_devices=1 if fake_cc else c.cores,
    )

    group0 = list(range(c.G))
    group1 = list(range(c.G, 2 * c.G))
    rgroups = [group0, group1]

    def all_gather(in_ap, out_t):
        if fake_cc:
            for g in range(c.G):
                nc.sync.dma_start(out=out_t.ap()[g], in_=in_ap)
        else:
            nc.gpsimd.collective_compute(
                "AllGather",
                mybir.AluOpType.bypass,
                replica_groups=rgroups,
                ins=[in_ap.opt()],
                outs=[out_t.ap().opt()],
            )

    def reduce_scatter(in_ap, out_ap):
        if fake_cc:
            nc.sync.dma_start(out=out_ap, in_=in_ap[0])
        else:
            nc.gpsimd.collective_compute(
                "ReduceScatter",
                mybir.AluOpType.add,
                replica_groups=rgroups,
                ins=[in_ap.opt()],
                outs=[out_ap.opt()],
            )

    scale = 1.0 / float(np.sqrt(c.dh))
    DHO = c.dh + 1  # dh + ones column

    # ---- DRAM parameters (host-prepped layouts, see shard_inputs) ----
    xT = nc.dram_tensor("xT", [P, c.KD, c.S_sl], BF, kind="ExternalInput")
    fqk = nc.dram_tensor("fqk", [P, c.KD, c.NR], BF, kind="ExternalInput")
    fv = nc.dram_tensor("fv", [P, c.KD, c.NR], BF, kind="ExternalInput")
    rqk = nc.dram_tensor("rqk", [P, c.KNR, c.COLS], BF, kind="ExternalInput")
    rv = nc.dram_tensor("rv", [P, c.KNR, c.COLS], BF, kind="ExternalInput")
    # W_O column shard: all 1024 rows x this core's 256 cols -> [P, KD, COLS]
    wo = nc.dram_tensor("wo", [P, c.KD, c.COLS], BF, kind="ExternalInput")
    # combine scalars (feature weights for this core's s-slice), f32
    wcomb = nc.dram_tensor("wcomb", [P, 3, c.ST_sl, c.N], F32, kind="ExternalInput")
    # restore weights, full S, bf16 (broadcast source): [3N, S]
    wsm = nc.dram_tensor("wsm", [3 * c.N, c.S], BF, kind="ExternalInput")
    out_d = nc.dram_tensor("out", [c.S, c.COLS], F32, kind="ExternalOutput")
    if debug_taps:
        dbg_qT = nc.dram_tensor("dbg_qT", [P, c.CT, c.S], F32, kind="ExternalOutput")
        dbg_kT = nc.dram_tensor("dbg_kT", [P, c.CT, c.S], F32, kind="ExternalOutput")
        dbg_v = nc.dram_tensor("dbg_v", [P, c.ST, c.Hpc * (c.dh + 1)], F32, kind="ExternalOutput")
        dbg_at = nc.dram_tensor("dbg_at", [P, c.CT, c.S], F32, kind="ExternalOutput")
        dbg_attn = nc.dram_tensor("dbg_attn", [P, c.ST, c.Hpc * c.dh], F32, kind="ExternalOutput")
        dbg_part = nc.dram_tensor("dbg_part", [c.G, c.S // 2, c.COLS], BF, kind="ExternalOutput")

    # collective bounce buffers
    hT_qk_in = nc.dram_tensor("hT_qk_in", [2, P, c.S_sl], BF)
    hT_v_in = nc.dram_tensor("hT_v_in", [P, c.S_sl], BF)
    hT_qk_out = nc.dram_tensor("hT_qk_out", [c.G, 2, P, c.S_sl], BF)
    hT_v_out = nc.dram_tensor("hT_v_out", [c.G, P, c.S_sl], BF)
    aT_in_l = [nc.dram_tensor(f"aT_in{ct}", [P, c.S], BF) for ct in range(c.CT)]
    aT_out_l = [
        nc.dram_tensor(f"aT_out{ct}", [c.G, P, c.S], BF) for ct in range(c.CT)
    ]

    from contextlib import ExitStack

    with tile.TileContext(nc) as tc, ExitStack() as stack:
        # ------- constants -------
        const_pool = stack.enter_context(tc.tile_pool(name="const", bufs=1))
        ident = const_pool.tile([P, P], BF)
        masks.make_identity(nc, ident[:])
        cmask = const_pool.tile([P, P], BF)
        masks.make_upper_triangular(nc, cmask[:], val=1.0, diag=True)

        # ------- long-lived SBUF residents -------
        res_pool = stack.enter_context(tc.tile_pool(name="residents", bufs=1))
        rqk_sb = res_pool.tile([P, c.KNR, c.COLS], BF)
        rv_sb = res_pool.tile([P, c.KNR, c.COLS], BF)
        wo_sb = res_pool.tile([P, c.KD, c.COLS], BF)
        wcomb_sb = res_pool.tile([P, 3, c.ST_sl, c.N], F32)
        nc.sync.dma_start(out=wcomb_sb[:], in_=wcomb[:])

        qT_ct = [
            res_pool.tile([P, c.S], BF, name=f"qT{i}") for i in range(c.CT)
        ]
        kT_ct = [
            res_pool.tile([P, c.S], BF, name=f"kT{i}") for i in range(c.CT)
        ]
        v_sb = res_pool.tile([P, c.ST, c.Hpc * DHO], BF)
        attn_sb = res_pool.tile([P, c.ST, c.Hpc * c.dh], BF)
        aT_sb = res_pool.tile([P, c.CT, c.S], BF)
        # rows [1, S] of the v restore weights (pool-broadcast source)
        wst_sb = res_pool.tile([c.N, c.S], BF)
        nc.sync.dma_start(out=wst_sb[:], in_=wsm.ap()[2 * c.N : 3 * c.N, :])

        # ones columns of v_sb
        v4 = v_sb[:].rearrange("p st (h x) -> p st h x", x=DHO)
        nc.gpsimd.memset(v4[:, :, :, c.dh : c.dh + 1], 1.0)

        # ================= Stage A: features on the s-slice =================
        with (
            tc.tile_pool(name="featA", bufs=2) as fpool,
            tc.tile_pool(name="featP", bufs=6, space="PSUM") as fps_pool,
            tc.tile_pool(name="featH", bufs=2) as hpool,
            tc.tile_pool(name="featHT", bufs=2, space="PSUM") as htps_pool,
        ):
            xT_sb = fpool.tile([P, c.KD, c.S_sl], BF, tag="xT", bufs=1)
            fqk_sb = fpool.tile([P, c.KD, c.NR], BF, tag="fqk", bufs=1)
            fv_sb = fpool.tile([P, c.KD, c.NR], BF, tag="fv", bufs=1)
            # per-k loads pipeline with the k-accumulation of the first
            # feature matmuls; fv is only needed in pass 2
            for k in range(c.KD):
                nc.sync.dma_start(out=xT_sb[:, k, :], in_=xT[:, k, :])
                nc.sync.dma_start(out=fqk_sb[:, k, :], in_=fqk[:, k, :])
            for k in range(c.KD):
                nc.sync.dma_start(out=fv_sb[:, k, :], in_=fv[:, k, :])

            # staging for h^T: copies land per s-tile, one DMA per tensor
            hT_stage = [
                hpool.tile([P, c.S_sl], BF, tag=f"hTs{t}", name=f"hT_stage{t}")
                for t in range(3)
            ]

            f_chunk = min(c.NR, 512)
            n_ch = _ceil_div(c.NR, f_chunk)
            n_per_ch = f_chunk // c.R

            def combine(t, st, ah_tiles):
                # h[s, r] = sum_n w[s, n] * all_h[s, n*R+r], 2-byte SBUF mode
                eng = nc.vector
                h_t = hpool.tile([P, c.R], BF, tag="hacc")
                for n in range(c.N):
                    ah = ah_tiles[n // n_per_ch]
                    src = ah[:, c.R * (n % n_per_ch) : c.R * (n % n_per_ch + 1)]
                    if n == 0:
                        eng.tensor_scalar(
                            out=h_t[:],
                            in0=src,
                            scalar1=wcomb_sb[:, t, st, 0:1],
                            scalar2=None,
                            op0=mybir.AluOpType.mult,
                        )
                    else:
                        eng.scalar_tensor_tensor(
                            out=h_t[:],
                            in0=src,
                            scalar=wcomb_sb[:, t, st, n : n + 1],
                            in1=h_t[:],
                            op0=mybir.AluOpType.mult,
                            op1=mybir.AluOpType.add,
                        )
                htp = htps_pool.tile([P, P], BF, tag="htp")
                nc.tensor.transpose(htp[:], h_t[:], ident[:])
                nc.scalar.copy(hT_stage[t][:, P * st : P * (st + 1)], htp[:, :])

            # pass 1: q/k features only, so their AllGather fires early;
            # pass 2 (v) runs under the q/k AllGather
            for pi, f_sb in ((0, fqk_sb), (1, fv_sb)):
                for st in range(c.ST_sl):
                    ah_tiles = []
                    for ch in range(n_ch):
                        ps = fps_pool.tile([P, f_chunk], F32, tag="feat")
                        lo = f_chunk * ch
                        hi = min(c.NR, lo + f_chunk)
                        for k in range(c.KD):
                            nc.tensor.matmul(
                                ps[:, 0 : hi - lo],
                                lhsT=xT_sb[:, k, P * st : P * (st + 1)],
                                rhs=f_sb[:, k, lo:hi],
                                start=(k == 0),
                                stop=(k == c.KD - 1),
                            )
                        ah = hpool.tile([P, f_chunk], BF, tag="ah", bufs=4)
                        nc.scalar.copy(ah[:], ps[:])
                        ah_tiles.append(ah)
                    if pi == 0:
                        combine(0, st, ah_tiles)
                        combine(1, st, ah_tiles)
                    else:
                        combine(2, st, ah_tiles)
                if pi == 0:
                    nc.sync.dma_start(out=hT_qk_in[0, :, :], in_=hT_stage[0][:])
                    nc.sync.dma_start(out=hT_qk_in[1, :, :], in_=hT_stage[1][:])
                    all_gather(hT_qk_in.ap()[:], hT_qk_out)
                else:
                    nc.sync.dma_start(out=hT_v_in.ap()[:], in_=hT_stage[2][:])
                    all_gather(hT_v_in.ap()[:], hT_v_out)

        # g tiles and q/k weight-row broadcasts: pools open once stage A's
        # SBUF is released; the broadcast DMAs run while the AllGather is in
        # flight.  One partition-broadcast DMA per tensor.
        g_pool = stack.enter_context(tc.tile_pool(name="g", bufs=16))
        g_tiles = {}
        wrep_stack = ExitStack()
        wrep_pool = wrep_stack.enter_context(tc.tile_pool(name="wrep", bufs=2))
        wr_full = {}
        for t in (0, 1):
            wrt = wrep_pool.tile([P, c.N, c.S], BF, tag="wrep", name=f"wr_t{t}")
            nc.sync.dma_start(
                out=wrt[:],
                in_=wsm.ap()[t * c.N : (t + 1) * c.N, :]
                .unsqueeze(0)
                .broadcast_to([P, c.N, c.S]),
            )
            wr_full[t] = wrt

        # deferred resident loads (consumed by restore / output projection)
        nc.sync.dma_start(out=rqk_sb[:], in_=rqk[:])
        nc.sync.dma_start(out=rv_sb[:], in_=rv[:])
        nc.sync.dma_start(out=wo_sb[:], in_=wo[:])

        # hT_full[r, g, t, s_in]  (s blocked by source rank g), batched DMAs
        hT_sb = res_pool.tile([P, c.G, 3, c.S_sl], BF)
        for t in range(2):
            nc.sync.dma_start(
                out=hT_sb[:, :, t, :],
                in_=hT_qk_out.ap()[:, t, :, :].rearrange("g p s -> p g s"),
            )
        nc.sync.dma_start(
            out=hT_sb[:, :, 2, :],
            in_=hT_v_out.ap().rearrange("g p s -> p g s"),
        )

        # ======= Stage C1: g tiles for q/k + Q^T/K^T restore (ct 0) =======
        n_sch = _ceil_div(c.S, 512)

        def qk_restore_chunk(pool, t, ct, ch, copy_eng):
            lo, hi = 512 * ch, min(c.S, 512 * ch + 512)
            dst = qT_ct[ct] if t == 0 else kT_ct[ct]
            rps = pool.tile([P, 512], F32, tag="rps")
            for n in range(c.KNR):
                nc.tensor.matmul(
                    rps[:, 0 : hi - lo],
                    lhsT=rqk_sb[:, n, P * ct : P * (ct + 1)],
                    rhs=g_tiles[(t, n)][:, lo:hi],
                    start=(n == 0),
                    stop=(n == c.KNR - 1),
                )
            copy_eng(dst[:, lo:hi], rps[:, 0 : hi - lo])

        with tc.tile_pool(name="rps0", bufs=2, space="PSUM") as rps0_pool:
            for t in (0, 1):
                for n in range(c.N):
                    g_tiles[(t, n)] = g_pool.tile(
                        [P, c.S], BF, tag="g", name=f"g_{t}_{n}"
                    )
            for ch in range(n_sch):
                lo, hi = 512 * ch, 512 * ch + 512
                for t in (0, 1):
                    for n in range(c.N):
                        eng = nc.vector if n < 6 else nc.gpsimd
                        eng.tensor_mul(
                            g_tiles[(t, n)][:, lo:hi],
                            hT_sb[:, ch, t, :],
                            wr_full[t][:, n, lo:hi],
                        )
                    qk_restore_chunk(rps0_pool, t, 0, ch, nc.scalar.copy)
        wrep_stack.close()  # q/k weight rows dead once g built

        # ================= Stage D: causal attention per head =================
        # per-j probs tiles sized to the causal width; j 0/1 double-buffered so
        # the next head's scores can start while this head's AVs drain
        pr_pool = stack.enter_context(tc.tile_pool(name="probs", bufs=1))
        asm_pool = stack.enter_context(tc.tile_pool(name="attn_small", bufs=4))
        atps_pool = stack.enter_context(
            tc.tile_pool(name="atps", bufs=1, space="PSUM")
        )
        av_pool = stack.enter_context(
            tc.tile_pool(name="avps", bufs=1, space="PSUM")
        )
        sps_stack = ExitStack()
        sps_pool = sps_stack.enter_context(
            tc.tile_pool(name="sps", bufs=2, space="PSUM")
        )
        late_stack = ExitStack()

        SCH = 1024  # scores chunk (2 PSUM banks); exp whole chunk

        def head_scores(h, js, probs):
            ct = (c.dh * h) // P
            off = (c.dh * h) % P
            for j in js:
                qlo = P * j
                qn = c.S - qlo
                pj = pr_pool.tile(
                    [P, qn], BF, tag=f"pj{j}", name=f"pj_{j}",
                    bufs=2 if j < 8 else 1,
                )
                probs.append(pj)
                for chx in range(_ceil_div(qn, SCH)):
                    lo = qlo + SCH * chx
                    hi = min(c.S, lo + SCH)
                    sps = sps_pool.tile([P, SCH], F32, tag="sps")
                    for sub in range(_ceil_div(hi - lo, 512)):
                        slo, shi = lo + 512 * sub, min(hi, lo + 512 * sub + 512)
                        nc.tensor.matmul(
                            sps[:, slo - lo : shi - lo],
                            lhsT=kT_ct[ct][off : off + c.dh, qlo : qlo + P],
                            rhs=qT_ct[ct][off : off + c.dh, slo:shi],
                            start=True,
                            stop=True,
                        )
                    nc.scalar.activation(
                        pj[:, lo - qlo : hi - qlo],
                        sps[:, 0 : hi - lo],
                        mybir.ActivationFunctionType.Exp,
                        scale=scale,
                    )
                # mask the diagonal tile (keep q >= k); Pool op frees DVE
                nc.gpsimd.tensor_mul(pj[:, 0:P], pj[:, 0:P], cmask[:])

        def head_av(h, probs, j, extra=None):
            av = av_pool.tile([P, DHO], F32, tag="av")
            for j2 in range(j + 1):
                nc.tensor.matmul(
                    av[:, :],
                    lhsT=probs[j2][:, P * (j - j2) : P * (j - j2) + P],
                    rhs=v_sb[:, j2, DHO * h : DHO * (h + 1)],
                    start=(j2 == 0),
                    stop=(j2 == j),
                )
            rec = asm_pool.tile([P, 1], F32, tag="rec")
            nc.vector.reciprocal(rec[:], av[:, c.dh : c.dh + 1])
            nc.vector.tensor_scalar(
                out=attn_sb[:, j, c.dh * h : c.dh * (h + 1)],
                in0=av[:, 0 : c.dh],
                scalar1=rec[:],
                scalar2=None,
                op0=mybir.AluOpType.mult,
            )

        # ---- pipelined attention schedule: each next head's first score
        # tiles are emitted before the current head's AVs (their pj tiles are
        # double-buffered) so the exp chain never starves at head boundaries;
        # V restore + QK ct-1 + g(v) fill the other engines under head 0 ----
        probs = [[] for _ in range(c.Hpc)]
        head_scores(0, range(c.ST), probs[0])

        # QK restore ct 1 (copies on DVE: Act is busy with exp)
        with tc.tile_pool(name="rps1", bufs=2, space="PSUM") as rps1_pool:
            for t in (0, 1):
                for ch in range(n_sch):
                    qk_restore_chunk(rps1_pool, t, 1, ch, nc.vector.tensor_copy)

        head_scores(1, range(0, 8), probs[1])

        # g tiles for v: rows broadcast on Pool, mults on DVE.  These reuse
        # the q g-tile slots, whose last readers are the ct-1 matmuls above —
        # so this section must stay after ct-1 in PE program order.
        with tc.tile_pool(name="wrb", bufs=3) as wrb_pool:
            wrbs = []
            for n in range(c.N):
                wrb = wrb_pool.tile([P, c.S], BF, tag="wrb", name=f"wrb{n}")
                nc.sync.dma_start(
                    out=wrb[:],
                    in_=wsm.ap()[2 * c.N + n : 2 * c.N + n + 1, :].broadcast_to(
                        [P, c.S]
                    ),
                )
                wrbs.append(wrb)
            for n in range(c.N):
                g_tiles[(2, n)] = g_pool.tile(
                    [P, c.S], BF, tag="g", name=f"g_2_{n}"
                )
                eng = nc.vector if n % 2 == 0 else nc.gpsimd
                eng.tensor_mul(
                    g_tiles[(2, n)][:].rearrange("p (g s) -> p g s", g=c.G),
                    hT_sb[:, :, 2, :],
                    wrbs[n][:].rearrange("p (g s) -> p g s", g=c.G),
                )

        # V restore interleaved with head-0 AV (scatter copies on DVE)
        with tc.tile_pool(name="vps", bufs=2, space="PSUM") as vps_pool:
            for st in range(c.ST):
                vps = vps_pool.tile([P, c.COLS], F32, tag="vps")
                for n in range(c.KNR):
                    nc.tensor.matmul(
                        vps[:, :],
                        lhsT=g_tiles[(2, n)][:, P * st : P * (st + 1)],
                        rhs=rv_sb[:, n, :],
                        start=(n == 0),
                        stop=(n == c.KNR - 1),
                    )
                nc.vector.tensor_copy(
                    v4[:, st, :, 0 : c.dh],
                    vps[:, :].rearrange("p (h x) -> p h x", x=c.dh),
                )
                head_av(0, probs[0], st)

        # late pools: output-projection partial staging
        def at_transpose(ct2, st):
            # attn^T tile for (ct2, st), staged for the all-gather
            atp = atps_pool.tile([P, P], BF, tag="atp")
            nc.tensor.transpose(
                atp[:, :], attn_sb[:, st, P * ct2 : P * (ct2 + 1)], ident[:]
            )
            nc.vector.tensor_copy(aT_sb[:, ct2, P * st : P * (st + 1)], atp[:, :])
            nc.sync.dma_start(
                out=aT_in_l[ct2].ap()[:, P * st : P * (st + 1)],
                in_=aT_sb[:, ct2, P * st : P * (st + 1)],
            )

        # ---- heads 1-3, software-pipelined; attn^T tiles ship per ct ----
        head_scores(1, range(8, c.ST), probs[1])
        head_scores(2, range(0, 8), probs[2])
        for j in range(c.ST):
            head_av(1, probs[1], j)
            at_transpose(0, j)
        all_gather(aT_in_l[0].ap()[:], aT_out_l[0])
        head_scores(2, range(8, c.ST), probs[2])
        head_scores(3, range(0, 8), probs[3])
        for j in range(c.ST):
            head_av(2, probs[2], j)
        head_scores(3, range(8, c.ST), probs[3])
        sps_stack.close()
        for j in range(c.ST):
            head_av(3, probs[3], j)
            at_transpose(1, j)
        all_gather(aT_in_l[1].ap()[:], aT_out_l[1])

        # ---- output projection: k-outer accumulation in arrival order so
        # matmuls start as soon as the first gathered column tile lands ----
        with (
            tc.tile_pool(name="ops", bufs=4, space="PSUM") as ops_pool,
            tc.tile_pool(name="osb", bufs=4) as osb_pool,
        ):
            # SBUF is tight: the gathered attn^T tiles land in buffers that
            # are dead by now (qT/kT after the last scores, aT/attn after the
            # transposes shipped)
            gdst = [
                qT_ct[0][:], qT_ct[1][:], kT_ct[0][:], kT_ct[1][:],
                aT_sb[:, 0, :], aT_sb[:, 1, :],
                attn_sb[:].rearrange("p a b -> p (a b)")[:, 0 : c.S],
                attn_sb[:].rearrange("p a b -> p (a b)")[:, c.S : 2 * c.S],
            ]
            arrival = []  # kd indices in DMA order
            for ct in range(c.CT):
                for g in range(c.G):
                    kd = g * c.CT + ct
                    arrival.append(kd)
                    nc.sync.dma_start(
                        out=gdst[kd], in_=aT_out_l[ct].ap()[g]
                    )
            kt_tot = c.G * c.CT
            GRP = 4  # s-tiles per pass (PSUM banks)
            for grp in range(_ceil_div(c.ST, GRP)):
                sts = range(GRP * grp, min(c.ST, GRP * (grp + 1)))
                ops_t = {
                    st: ops_pool.tile(
                        [P, c.COLS], F32, tag="ops", name=f"ops_{st}"
                    )
                    for st in sts
                }
                for ki, kd in enumerate(arrival):
                    for st in sts:
                        nc.tensor.matmul(
                            ops_t[st][:, :],
                            lhsT=gdst[kd][:, P * st : P * (st + 1)],
                            rhs=wo_sb[:, kd, :],
                            start=(ki == 0),
                            stop=(ki == kt_tot - 1),
                        )
                for st in sts:
                    osb = osb_pool.tile([P, c.COLS], F32, tag="osb")
                    nc.scalar.copy(osb[:], ops_t[st][:, :])
                    nc.sync.dma_start(
                        out=out_d.ap()[P * st : P * (st + 1), :], in_=osb[:]
                    )
        late_stack.close()

    nc.compile()
    return nc


# ---------------------------------------------------------------------------
# Host-side sharding / gathering
# ---------------------------------------------------------------------------


def shard_inputs(
    inputs: dict,
    cfg: Cfg = FULL,
) -> list[dict]:
    c = cfg
    x = np.asarray(inputs["x"], np.float32)
    fqk_n = np.asarray(inputs["f_qk_neurons"], np.float32)
    fv_n = np.asarray(inputs["f_v_neurons"], np.float32)
    rqk_n = np.asarray(inputs["r_qk_neurons"], np.float32)
    rv_n = np.asarray(inputs["r_v_neurons"], np.float32)
    w_o = np.asarray(inputs["W_O"], np.float32)

    def tile_p(a, kt):  # [D, M] -> [P, kt, M]
        d, m = a.shape
        assert d == kt * P
        return np.ascontiguousarray(a.reshape(kt, P, m).transpose(1, 0, 2))

    # [N, D, R] -> [D, N*R]
    f_qk_flat = fqk_n.transpose(1, 0, 2).reshape(c.D, c.NR)
    f_v_flat = fv_n.transpose(1, 0, 2).reshape(c.D, c.NR)
    # [N, R, D] -> [N*R, D]
    r_qk_flat = rqk_n.reshape(c.NR, c.D)
    r_v_flat = rv_n.reshape(c.NR, c.D)

    in_maps = []
    for core in range(c.cores):
        b, g = core // c.G, core % c.G
        sl = slice(c.S_sl * g, c.S_sl * (g + 1))
        cols = slice(c.COLS * g, c.COLS * (g + 1))
        rows = slice(c.COLS * g, c.COLS * (g + 1))

        xT = x[b].T[:, sl]  # [D, S_sl]

        wq = np.asarray(inputs["fqk_weights_Q"], np.float32)[b, sl]  # [S_sl, N]
        wk = np.asarray(inputs["fqk_weights_K"], np.float32)[b, sl]
        wv = np.asarray(inputs["fv_weights"], np.float32)[b, sl]
        wcomb = np.stack([wq, wk, wv], 0)  # [3, S_sl, N]
        wcomb = np.ascontiguousarray(
            wcomb.reshape(3, c.ST_sl, P, c.N).transpose(2, 0, 1, 3)
        )  # [P, 3, ST_sl, N]

        wsm = np.stack(
            [
                np.asarray(inputs["rqk_weights_Q"], np.float32)[b].T,
                np.asarray(inputs["rqk_weights_K"], np.float32)[b].T,
                np.asarray(inputs["rv_weights"], np.float32)[b].T,
            ],
            0,
        ).reshape(3 * c.N, c.S)  # [3N, S]

        m = {
            "xT": tile_p(xT, c.KD).astype(BF16),
            "fqk": tile_p(f_qk_flat, c.KD).astype(BF16),
            "fv": tile_p(f_v_flat, c.KD).astype(BF16),
            "rqk": tile_p(r_qk_flat[:, cols], c.KNR).astype(BF16),
            "rv": tile_p(r_v_flat[:, cols], c.KNR).astype(BF16),
            "wo": tile_p(w_o[:, cols], c.KD).astype(BF16),
            "wcomb": wcomb.astype(np.float32),
            "wsm": wsm.astype(BF16),
        }
        in_maps.append(m)
    return in_maps


def gather_output(results: list[dict], cfg: Cfg = FULL) -> np.ndarray:
    c = cfg
    out = np.empty((c.B, c.S, c.D), np.float32)
    for core in range(c.cores):
        b, g = core // c.G, core % c.G
        out[b, :, c.COLS * g : c.COLS * (g + 1)] = np.asarray(
            results[core]["out"], np.float32
        )
    return out


_NC_CACHE = {}


def get_nc(cfg: Cfg = FULL) -> bacc.Bacc:
    if cfg not in _NC_CACHE:
        _NC_CACHE[cfg] = build_nc(cfg)
    return _NC_CACHE[cfg]


def kernel(**inputs) -> np.ndarray:
    cfg = FULL
    nc = get_nc(cfg)
    in_maps = shard_inputs(inputs, cfg)
    res = run_bass_kernel_spmd(nc, in_maps, core_ids=list(range(cfg.cores)))
    return gather_output(res.results, cfg)


# revision 72
# speedup vs baseline: 1.1214x; 1.1088x over previous
"""Trainium2 Bass kernel for nn_AttentionCircuit (neuron-mixture attention).

Self-contained: accepts FULL inputs, shards across 8 NeuronCores, runs a
Bass/Tile SPMD kernel, gathers the full output.

Sharding: core c = (b, g) with b = c // 4 (batch), g = c % 4 (head-group of
4 heads = 256 channels).  Features are sequence-split within each batch
group and all-gathered; restore + attention are head-group-parallel; the
output projection uses a column shard of W_O after all-gathering the
attention output (transposed layout).  All TensorEngine compute in bf16,
f32 I/O and PSUM accumulation.
"""

import sys

for _p in ("/opt/trn_rl_repo",):
    if _p not in sys.path:
        sys.path.append(_p)

import numpy as np
from dataclasses import dataclass

import concourse.bass as bass
import concourse.bacc as bacc
import concourse.mybir as mybir
import concourse.tile as tile
from concourse import masks
from concourse.bass_utils import run_bass_kernel_spmd

try:
    import ml_dtypes

    BF16 = ml_dtypes.bfloat16
except ImportError:  # pragma: no cover
    BF16 = np.float32


def _install_neff_disk_cache():
    """Cache walrus BIR->NEFF compiles on disk (keyed by BIR bytes) so
    repeated runs of the identical graph skip the multi-minute compile."""
    import hashlib, os, tempfile
    from concourse import bass2jax

    if getattr(bass2jax, "_ant_neff_cache_installed", False):
        return
    orig = bass2jax.compile_bir_kernel
    cache_dir = os.path.join(tempfile.gettempdir(), "bass_neff_cache")
    os.makedirs(cache_dir, exist_ok=True)

    def cached(bir_json, tmpdir, neff_name="file.neff"):
        key = hashlib.sha256(bir_json).hexdigest()
        path = os.path.join(cache_dir, key + ".neff")
        dst = os.path.join(tmpdir, neff_name)
        if os.path.exists(path):
            import shutil

            shutil.copy(path, dst)
            return dst
        neff = orig(bir_json, tmpdir, neff_name=neff_name)
        try:
            import shutil

            shutil.copy(neff, path)
        except OSError:
            pass
        return neff

    bass2jax.compile_bir_kernel = cached
    bass2jax._ant_neff_cache_installed = True


_install_neff_disk_cache()

F32 = mybir.dt.float32
BF = mybir.dt.bfloat16
P = 128  # partitions


@dataclass(frozen=True)
class Cfg:
    B: int = 2
    S: int = 2048
    D: int = 1024
    R: int = 128
    N: int = 8
    H: int = 16
    cores: int = 8

    @property
    def G(self):  # cores per batch == head groups
        return self.cores // self.B

    @property
    def S_sl(self):  # sequence slice per core (feature stage)
        return self.S // self.G

    @property
    def COLS(self):  # channel columns per core
        return self.D // self.G

    @property
    def Hpc(self):  # heads per core
        return self.H // self.G

    @property
    def dh(self):
        return self.D // self.H

    @property
    def KD(self):  # k-tiles over D
        return self.D // P

    @property
    def NR(self):
        return self.N * self.R

    @property
    def KNR(self):  # k-tiles over N*R
        return self.NR // P

    @property
    def ST(self):  # s-tiles over full S
        return self.S // P

    @property
    def ST_sl(self):  # s-tiles over S slice
        return self.S_sl // P

    @property
    def CT(self):  # 128-col tiles over COLS
        return (self.COLS + P - 1) // P


FULL = Cfg()


def _ceil_div(a, b):
    return (a + b - 1) // b


def build_nc(cfg: Cfg = FULL, fake_cc: bool = False) -> bacc.Bacc:
    """Build + compile the SPMD graph (identical on every core).

    fake_cc=True replaces collectives with local DMA replication (wrong
    results) so the single-core TimelineSim can cost-model the kernel.
    """
    c = cfg
    assert c.R == P and c.D % P == 0 and c.S_sl % P == 0
    assert P % c.dh == 0 and c.COLS % c.dh == 0 and c.COLS % P == 0

    nc = bacc.Bacc(
        "TRN2",
        target_bir_lowering=False,
        debug=False,
        num_devices=1 if fake_cc else c.cores,
    )

    def all_gather(in_t, out_t):
        if fake_cc:
            for g in range(c.G):
                nc.sync.dma_start(out=out_t.ap()[g], in_=in_t.ap()[:])
        else:
            nc.gpsimd.collective_compute(
                "AllGather",
                mybir.AluOpType.bypass,
                replica_groups=rgroups,
                ins=[in_t.ap().opt()],
                outs=[out_t.ap().opt()],
            )

    # ---- DRAM parameters (host-prepped layouts, see shard_inputs) ----
    xT = nc.dram_tensor("xT", [P, c.KD, c.S_sl], BF, kind="ExternalInput")
    fqk = nc.dram_tensor("fqk", [P, c.KD, c.NR], BF, kind="ExternalInput")
    fv = nc.dram_tensor("fv", [P, c.KD, c.NR], BF, kind="ExternalInput")
    rqk = nc.dram_tensor("rqk", [P, c.KNR, c.COLS], BF, kind="ExternalInput")
    rv = nc.dram_tensor("rv", [P, c.KNR, c.COLS], BF, kind="ExternalInput")
    wo = nc.dram_tensor("wo", [P, c.KD, c.COLS], BF, kind="ExternalInput")
    # combine scalars (feature weights for this core's s-slice), f32
    wcomb = nc.dram_tensor("wcomb", [P, 3, c.ST_sl, c.N], F32, kind="ExternalInput")
    # restore weights, full S, bf16 (broadcast source): [3, N, S]
    wsm = nc.dram_tensor("wsm", [3 * c.N, c.S], BF, kind="ExternalInput")
    out_d = nc.dram_tensor("out", [c.S, c.COLS], F32, kind="ExternalOutput")

    group0 = list(range(c.G))
    group1 = list(range(c.G, 2 * c.G))
    rgroups = [group0, group1]

    scale = 1.0 / float(np.sqrt(c.dh))
    DHO = c.dh + 1  # dh + ones column

    from contextlib import ExitStack

    with tile.TileContext(nc) as tc, ExitStack() as stack:
        # ------- constants -------
        const_pool = stack.enter_context(tc.tile_pool(name="const", bufs=1))
        ident = const_pool.tile([P, P], BF)
        masks.make_identity(nc, ident[:])
        cmask = const_pool.tile([P, P], BF)
        masks.make_upper_triangular(nc, cmask[:], val=1.0, diag=True)

        # ------- long-lived SBUF residents (DMAs for stage-C/F consumers are
        # emitted after stage A so they don't delay the critical xT/f loads)
        res_pool = stack.enter_context(tc.tile_pool(name="residents", bufs=1))
        rqk_sb = res_pool.tile([P, c.KNR, c.COLS], BF)
        rv_sb = res_pool.tile([P, c.KNR, c.COLS], BF)
        wo_sb = res_pool.tile([P, c.KD, c.COLS], BF)
        wcomb_sb = res_pool.tile([P, 3, c.ST_sl, c.N], F32)
        nc.sync.dma_start(out=wcomb_sb[:], in_=wcomb[:])

        qT_sb = res_pool.tile([P, c.CT, c.S], BF)
        kT_sb = res_pool.tile([P, c.CT, c.S], BF)
        v_sb = res_pool.tile([P, c.ST, c.Hpc * DHO], BF)
        attn_sb = res_pool.tile([P, c.ST, c.Hpc * c.dh], BF)

        # ones columns of v_sb
        v4 = v_sb[:].rearrange("p st (h x) -> p st h x", x=DHO)
        nc.gpsimd.memset(v4[:, :, :, c.dh : c.dh + 1], 1.0)

        # DRAM bounce buffers for collectives (aT gathered per column tile so
        # the first collective overlaps attention of the remaining heads)
        hT_in = nc.dram_tensor("hT_in", [3, P, c.S_sl], BF)
        hT_out = nc.dram_tensor("hT_out", [c.G, 3, P, c.S_sl], BF)
        aT_in_l = [
            nc.dram_tensor(f"aT_in{ct}", [P, c.S], BF) for ct in range(c.CT)
        ]
        aT_out_l = [
            nc.dram_tensor(f"aT_out{ct}", [c.G, P, c.S], BF) for ct in range(c.CT)
        ]

        # wrep/g pools span stages A+C: the broadcasts are emitted during
        # stage A so they complete on the DMA queues before restore needs them
        cstack = ExitStack()
        wrep_pool = cstack.enter_context(
            tc.tile_pool(name="wrep", bufs=c.N + 2)
        )
        wr_tiles = {}
        g_tiles = {}

        # ================= Stage A: features on the s-slice =================
        with (
            tc.tile_pool(name="featA", bufs=2) as fpool,
            tc.tile_pool(name="featP", bufs=6, space="PSUM") as fps_pool,
            tc.tile_pool(name="featH", bufs=2) as hpool,
            tc.tile_pool(name="featHT", bufs=2, space="PSUM") as htps_pool,
        ):
            xT_sb = fpool.tile([P, c.KD, c.S_sl], BF, tag="xT", bufs=1)
            fqk_sb = fpool.tile([P, c.KD, c.NR], BF, tag="fqk", bufs=1)
            fv_sb = fpool.tile([P, c.KD, c.NR], BF, tag="fv", bufs=1)
            for k in range(c.KD):
                nc.sync.dma_start(out=xT_sb[:, k, :], in_=xT[:, k, :])
                nc.sync.dma_start(out=fqk_sb[:, k, :], in_=fqk[:, k, :])
                nc.sync.dma_start(out=fv_sb[:, k, :], in_=fv[:, k, :])

            # W_rep broadcasts: V's go through the idle Pool engine during
            # stage A (keeps the DMA queues clear); q/k replicate via DMA
            # during stage C when the queues have slack.
            def make_wr(t, use_pool=False):
                for n in range(c.N):
                    row = t * c.N + n
                    wr = wrep_pool.tile([P, c.S], BF, tag="wrep", name=f"wr_{row}")
                    if use_pool:
                        wst = wrep_pool.tile(
                            [1, c.S], BF, tag="wstage", bufs=2, name=f"wst_{row}"
                        )
                        nc.sync.dma_start(out=wst[:], in_=wsm[row : row + 1, :])
                        nc.gpsimd.partition_broadcast(wr[:], wst[0:1, :])
                    else:
                        nc.sync.dma_start(
                            out=wr[:],
                            in_=wsm.ap()[row : row + 1, :].broadcast_to([P, c.S]),
                        )
                    wr_tiles[(t, n)] = wr

            make_wr(2, use_pool=True)

            f_chunk = min(c.NR, 512)
            n_ch = _ceil_div(c.NR, f_chunk)
            n_per_ch = f_chunk // c.R
            for st in range(c.ST_sl):
                ps_tiles = {}
                for pi, f_sb in ((0, fqk_sb), (1, fv_sb)):
                    for ch in range(n_ch):
                        ps = fps_pool.tile([P, f_chunk], F32, tag="feat")
                        ps_tiles[(pi, ch)] = ps
                        lo = f_chunk * ch
                        hi = min(c.NR, lo + f_chunk)
                        for k in range(c.KD):
                            nc.tensor.matmul(
                                ps[:, 0 : hi - lo],
                                lhsT=xT_sb[:, k, P * st : P * (st + 1)],
                                rhs=f_sb[:, k, lo:hi],
                                start=(k == 0),
                                stop=(k == c.KD - 1),
                            )
                # copy all_h PSUM -> SBUF bf16 once (cheap), then combine in
                # 2-byte SBUF mode: h[s, r] = sum_n w[s, n] * all_h[s, n*R+r]
                ah_tiles = {}
                for pi in (0, 1):
                    for ch in range(n_ch):
                        ah = hpool.tile([P, f_chunk], BF, tag="ah", bufs=4)
                        nc.scalar.copy(ah[:], ps_tiles[(pi, ch)][:])
                        ah_tiles[(pi, ch)] = ah
                for t, pi in ((0, 0), (1, 0), (2, 1)):
                    h_t = hpool.tile([P, c.R], BF, tag="hacc")
                    for n in range(c.N):
                        ah = ah_tiles[(pi, n // n_per_ch)]
                        src = ah[:, c.R * (n % n_per_ch) : c.R * (n % n_per_ch + 1)]
                        if n == 0:
                            nc.vector.tensor_scalar(
                                out=h_t[:],
                                in0=src,
                                scalar1=wcomb_sb[:, t, st, 0:1],
                                scalar2=None,
                                op0=mybir.AluOpType.mult,
                            )
                        else:
                            nc.vector.scalar_tensor_tensor(
                                out=h_t[:],
                                in0=src,
                                scalar=wcomb_sb[:, t, st, n : n + 1],
                                in1=h_t[:],
                                op0=mybir.AluOpType.mult,
                                op1=mybir.AluOpType.add,
                            )
                    htp = htps_pool.tile([P, P], BF, tag="htp")
                    nc.tensor.transpose(htp[:], h_t[:], ident[:])
                    hT_sl = hpool.tile([P, P], BF, tag="hT", bufs=3)
                    nc.scalar.copy(hT_sl[:], htp[:, :])
                    nc.sync.dma_start(
                        out=hT_in[t, :, P * st : P * (st + 1)], in_=hT_sl[:]
                    )

            # ---- AllGather h^T across the batch group ----
            all_gather(hT_in, hT_out)

        # deferred resident loads (consumed by stage C/F)
        nc.sync.dma_start(out=rv_sb[:], in_=rv[:])
        nc.sync.dma_start(out=rqk_sb[:], in_=rqk[:])
        nc.sync.dma_start(out=wo_sb[:], in_=wo[:])

        # hT_full[r, t, g, s_in]  (s blocked by source rank g), per-block DMAs
        hT_sb = res_pool.tile([P, 3, c.G, c.S_sl], BF)
        for t in range(3):
            for g in range(c.G):
                nc.sync.dma_start(
                    out=hT_sb[:, t, g, :], in_=hT_out.ap()[g, t, :, :]
                )

        # ============ Stage C: restore projections (V, then Q^T/K^T) ============
        # g tiles are per-(tensor, n); the multiplies are chunked per source
        # block and split across DVE and GpSimd so the PE can start each
        # accumulation as soon as possible.
        g_pool = cstack.enter_context(tc.tile_pool(name="g", bufs=2 * c.N + 2))
        n_sch = _ceil_div(c.S, 512)

        def make_g(t):
            for n in range(c.N):
                row = t * c.N + n
                g_t = g_pool.tile([P, c.S], BF, tag="g", name=f"g_{row}")
                g_tiles[(t, n)] = g_t
            for blk in range(c.G):
                lo, hi = c.S_sl * blk, c.S_sl * (blk + 1)
                for n in range(c.N):
                    eng = nc.vector
                    eng.tensor_mul(
                        g_tiles[(t, n)][:, lo:hi],
                        hT_sb[:, t, blk, :],
                        wr_tiles[(t, n)][:, lo:hi],
                    )

        # ---- V ----
        make_g(2)
        with tc.tile_pool(name="vps", bufs=4, space="PSUM") as vps_pool:
            for st in range(c.ST):
                vps = vps_pool.tile([P, c.COLS], F32, tag="vps")
                for n in range(c.KNR):
                    nc.tensor.matmul(
                        vps[:, :],
                        lhsT=g_tiles[(2, n)][:, P * st : P * (st + 1)],
                        rhs=rv_sb[:, n, :],
                        start=(n == 0),
                        stop=(n == c.KNR - 1),
                    )
                # scatter into per-head blocks of v_sb (stride dh+1)
                nc.scalar.copy(
                    v4[:, st, :, 0 : c.dh],
                    vps[:, :].rearrange("p (h x) -> p h x", x=c.dh),
                )

        # ---- Q^T / K^T, column tile ct=0 first, then ct=1 ----
        make_wr(0)
        make_g(0)
        make_wr(1)
        make_g(1)
        with tc.tile_pool(name="rps", bufs=2, space="PSUM") as rps_pool:
            for ct in range(c.CT):
                for t, dst, r_sb in ((0, qT_sb, rqk_sb), (1, kT_sb, rqk_sb)):
                    pt = min(P, c.COLS - P * ct)
                    rps = rps_pool.tile([P, c.S], F32, tag="rps")
                    for ch in range(n_sch):
                        lo, hi = 512 * ch, min(c.S, 512 * ch + 512)
                        for n in range(c.KNR):
                            nc.tensor.matmul(
                                rps[:pt, lo:hi],
                                lhsT=r_sb[:, n, P * ct : P * ct + pt],
                                rhs=g_tiles[(t, n)][:, lo:hi],
                                start=(n == 0),
                                stop=(n == c.KNR - 1),
                            )
                    nc.scalar.copy(dst[:pt, ct, :], rps[:pt, :])
        cstack.close()

        # ================= Stage D: causal attention per head =================
        with (
            tc.tile_pool(name="probs", bufs=c.ST + 2) as pr_pool,
            tc.tile_pool(name="sps", bufs=3, space="PSUM") as sps_pool,
            tc.tile_pool(name="avps", bufs=1, space="PSUM") as av_pool,
            tc.tile_pool(name="attn_small", bufs=4) as asm_pool,
            tc.tile_pool(name="atps", bufs=1, space="PSUM") as atps_pool,
        ):
            for h in range(c.Hpc):
                ct = (c.dh * h) // P
                off = (c.dh * h) % P
                probs = []
                for j in range(c.ST):
                    qlo = P * j
                    qn = c.S - qlo
                    pj = pr_pool.tile([P, c.S], BF, tag="probs")
                    probs.append(pj)
                    SCH = 1024  # scores chunk (2 PSUM banks); exp whole chunk
                    for ch in range(_ceil_div(qn, SCH)):
                        lo = qlo + SCH * ch
                        hi = min(c.S, lo + SCH)
                        sps = sps_pool.tile([P, SCH], F32, tag="sps")
                        for sub in range(_ceil_div(hi - lo, 512)):
                            slo, shi = lo + 512 * sub, min(hi, lo + 512 * sub + 512)
                            nc.tensor.matmul(
                                sps[:, slo - lo : shi - lo],
                                lhsT=kT_sb[off : off + c.dh, ct, qlo : qlo + P],
                                rhs=qT_sb[off : off + c.dh, ct, slo:shi],
                                start=True,
                                stop=True,
                            )
                        nc.scalar.activation(
                            pj[:, lo - qlo : hi - qlo],
                            sps[:, 0 : hi - lo],
                            mybir.ActivationFunctionType.Exp,
                            scale=scale,
                        )
                    # mask the diagonal tile (keep q >= k)
                    nc.vector.tensor_mul(pj[:, 0:P], pj[:, 0:P], cmask[:])
                    # AV for q-tile j: k-tiles 0..j
                    av = av_pool.tile([P, DHO], F32, tag="av")
                    for j2 in range(j + 1):
                        nc.tensor.matmul(
                            av[:, :],
                            lhsT=probs[j2][:, P * (j - j2) : P * (j - j2) + P],
                            rhs=v_sb[:, j2, DHO * h : DHO * (h + 1)],
                            start=(j2 == 0),
                            stop=(j2 == j),
                        )
                    rec = asm_pool.tile([P, 1], F32, tag="rec")
                    nc.vector.reciprocal(rec[:], av[:, c.dh : c.dh + 1])
                    nc.vector.tensor_scalar(
                        out=attn_sb[:, j, c.dh * h : c.dh * (h + 1)],
                        in0=av[:, 0 : c.dh],
                        scalar1=rec[:],
                        scalar2=None,
                        op0=mybir.AluOpType.mult,
                    )

                # once both heads of a column tile are done: transpose that
                # tile, ship it, and launch its all-gather (overlaps with the
                # remaining heads' attention)
                if (h + 1) * c.dh % P == 0:
                    ct2 = ((h + 1) * c.dh) // P - 1
                    for st in range(c.ST):
                        atp = atps_pool.tile([P, P], BF, tag="atp")
                        nc.tensor.transpose(
                            atp[:, :],
                            attn_sb[:, st, P * ct2 : P * (ct2 + 1)],
                            ident[:],
                        )
                        at_sl = asm_pool.tile([P, P], BF, tag="at_sl")
                        nc.vector.tensor_copy(at_sl[:, :], atp[:, :])
                        nc.sync.dma_start(
                            out=aT_in_l[ct2][:, P * st : P * (st + 1)],
                            in_=at_sl[:, :],
                        )
                    all_gather(aT_in_l[ct2], aT_out_l[ct2])

        # ================= Stage F: output projection =================
        # k-outer accumulation in arrival order (ct-major) so matmuls start
        # as soon as the first gathered column tile lands.
        with (
            tc.tile_pool(name="aT_full", bufs=1) as atf_pool,
            tc.tile_pool(name="ops", bufs=8, space="PSUM") as ops_pool,
            tc.tile_pool(name="osb", bufs=4) as osb_pool,
        ):
            aTf_sb = atf_pool.tile([P, c.G * c.CT, c.S], BF)
            arrival = []  # kd indices in DMA order
            for ct in range(c.CT):
                for g in range(c.G):
                    kd = g * c.CT + ct
                    arrival.append(kd)
                    nc.sync.dma_start(
                        out=aTf_sb[:, kd, :], in_=aT_out_l[ct].ap()[g]
                    )
            kt_tot = c.G * c.CT  # == KD when COLS*G == D
            GRP = 4  # st-tiles per pass (PSUM banks)
            for grp in range(_ceil_div(c.ST, GRP)):
                sts = range(GRP * grp, min(c.ST, GRP * (grp + 1)))
                ops_t = {
                    st: ops_pool.tile([P, c.COLS], F32, tag="ops", name=f"ops_{st}")
                    for st in sts
                }
                for ki, kd in enumerate(arrival):
                    for st in sts:
                        nc.tensor.matmul(
                            ops_t[st][:, :],
                            lhsT=aTf_sb[:, kd, P * st : P * (st + 1)],
                            rhs=wo_sb[:, kd, :],
                            start=(ki == 0),
                            stop=(ki == kt_tot - 1),
                        )
                for st in sts:
                    osb = osb_pool.tile([P, c.COLS], F32, tag="osb")
                    nc.scalar.copy(osb[:], ops_t[st][:, :])
                    nc.sync.dma_start(
                        out=out_d.ap()[P * st : P * (st + 1), :], in_=osb[:]
                    )

    nc.compile()
    return nc


# ---------------------------------------------------------------------------
# Host-side sharding / gathering
# ---------------------------------------------------------------------------


def shard_inputs(
    inputs: dict,
    cfg: Cfg = FULL,
) -> list[dict]:
    c = cfg
    x = np.asarray(inputs["x"], np.float32)
    fqk_n = np.asarray(inputs["f_qk_neurons"], np.float32)
    fv_n = np.asarray(inputs["f_v_neurons"], np.float32)
    rqk_n = np.asarray(inputs["r_qk_neurons"], np.float32)
    rv_n = np.asarray(inputs["r_v_neurons"], np.float32)
    w_o = np.asarray(inputs["W_O"], np.float32)

    def tile_p(a, kt):  # [D, M] -> [P, kt, M]
        d, m = a.shape
        assert d == kt * P
        return np.ascontiguousarray(a.reshape(kt, P, m).transpose(1, 0, 2))

    # [N, D, R] -> [D, N*R]
    f_qk_flat = fqk_n.transpose(1, 0, 2).reshape(c.D, c.NR)
    f_v_flat = fv_n.transpose(1, 0, 2).reshape(c.D, c.NR)
    # [N, R, D] -> [N*R, D]
    r_qk_flat = rqk_n.reshape(c.NR, c.D)
    r_v_flat = rv_n.reshape(c.NR, c.D)

    in_maps = []
    for core in range(c.cores):
        b, g = core // c.G, core % c.G
        sl = slice(c.S_sl * g, c.S_sl * (g + 1))
        cols = slice(c.COLS * g, c.COLS * (g + 1))

        xT = x[b].T[:, sl]  # [D, S_sl]

        wq = np.asarray(inputs["fqk_weights_Q"], np.float32)[b, sl]  # [S_sl, N]
        wk = np.asarray(inputs["fqk_weights_K"], np.float32)[b, sl]
        wv = np.asarray(inputs["fv_weights"], np.float32)[b, sl]
        wcomb = np.stack([wq, wk, wv], 0)  # [3, S_sl, N]
        wcomb = np.ascontiguousarray(
            wcomb.reshape(3, c.ST_sl, P, c.N).transpose(2, 0, 1, 3)
        )  # [P, 3, ST_sl, N]

        wsm = np.stack(
            [
                np.asarray(inputs["rqk_weights_Q"], np.float32)[b].T,
                np.asarray(inputs["rqk_weights_K"], np.float32)[b].T,
                np.asarray(inputs["rv_weights"], np.float32)[b].T,
            ],
            0,
        ).reshape(3 * c.N, c.S)  # [3N, S]

        m = {
            "xT": tile_p(xT, c.KD).astype(BF16),
            "fqk": tile_p(f_qk_flat, c.KD).astype(BF16),
            "fv": tile_p(f_v_flat, c.KD).astype(BF16),
            "rqk": tile_p(r_qk_flat[:, cols], c.KNR).astype(BF16),
            "rv": tile_p(r_v_flat[:, cols], c.KNR).astype(BF16),
            "wo": tile_p(w_o[:, cols], c.KD).astype(BF16),
            "wcomb": wcomb.astype(np.float32),
            "wsm": wsm.astype(BF16),
        }
        in_maps.append(m)
    return in_maps


def gather_output(results: list[dict], cfg: Cfg = FULL) -> np.ndarray:
    c = cfg
    out = np.empty((c.B, c.S, c.D), np.float32)
    for core in range(c.cores):
        b, g = core // c.G, core % c.G
        out[b, :, c.COLS * g : c.COLS * (g + 1)] = np.asarray(
            results[core]["out"], np.float32
        )
    return out


_NC_CACHE = {}


def get_nc(cfg: Cfg = FULL) -> bacc.Bacc:
    if cfg not in _NC_CACHE:
        _NC_CACHE[cfg] = build_nc(cfg)
    return _NC_CACHE[cfg]


def kernel(**inputs) -> np.ndarray:
    cfg = FULL
    nc = get_nc(cfg)
    in_maps = shard_inputs(inputs, cfg)
    res = run_bass_kernel_spmd(nc, in_maps, core_ids=list(range(cfg.cores)))
    return gather_output(res.results, cfg)

